# revision 32
# baseline (speedup 1.0000x reference)
"""Trainium2 Bass kernel for nn_DistanceModule.

Computes, for h [4,512,64], W [64,64], b/gamma/beta [64]:
    x = LayerNorm(ReLU(h @ W.T + b))          # [B,N,C]
    D[b,i,j,c] = x[b,i,c] * x[b,j,c]
    out = softmax(D, axis=-1)                 # [B,N,N,C] f32 (256 MB)

Sharding: 2048 (b,i) rows split across 8 cores -> 256 rows/core
(core k: batch b=k//2, i in [256*(k%2), 256*(k%2)+256)). Each core
computes x[b] on-chip, then streams its [256, 512, 64] output slice.
All cores run one identical NEFF; per-core behavior comes only from
per-core input slices (hT = h[b].T, hTi = h[b, i0:i0+256].T).

Per-core pipeline, per (i-tile, j-chunk), all engines overlapped:
  PE     : one K=128 bf16 matmul per channel c broadcasts xT row c
           across 128 partitions into PSUM. The K axis stacks an exact
           hi/lo bf16 split of xT (x = hi + lo to ~2^-17) against a
           doubled 0/1 selector, so fp32 accuracy is reproduced at bf16
           matmul speed in a single pass.
  ScalarE: activation(Exp, scale=x_i[:,c]) reads the PSUM broadcast and
           fuses the x_i*x_j multiply into the exp via the per-partition
           scale operand -- one FD=jw instruction per (i-tile, c).
  VectorE: segmented reduce_sum over c (axis=X on the [128, j, c] view),
           reciprocal, then in-place normalize multiply against a
           stride-0-broadcast reciprocal AP. This engine is the
           critical path (~1 elem/lane/cycle for reduce and multiply).
  DMA    : normalize runs in j-quarters, each immediately stored with a
           contiguous 128-partition HWDGE DMA (64KB/partition rows).

Chunk widths (224/288) keep ScalarE's per-instruction overhead (~400
cycles, from the per-partition bias+scale preloads) balanced against
VectorE's chunk time, and a narrow first/last chunk trims the pipeline
fill/drain.

Softmax is computed without max-subtraction: LayerNorm bounds |x| by
sqrt(C-1) ~= 7.94, so logits <= 63 and exp <= 2.4e27 < f32 max.
Measured: ~228 us HW exec, rel err ~5e-6 vs the f32 reference.
"""

import numpy as np

import concourse.bacc as bacc
import concourse.bass as bass
import concourse.mybir as mybir
import concourse.tile as tile
from concourse.bass_utils import run_bass_kernel_spmd

B, N, C = 4, 512, 64
NCORES = 8
ROWS = 256          # (b,i) rows per core
JBLK = 256          # j-block width
EPS = 1e-5
F32 = mybir.dt.float32
BF16 = mybir.dt.bfloat16

_CACHE = {}


def _build_program():
    nc = bacc.Bacc(
        "TRN2",
        target_bir_lowering=False,
        debug=False,
        enable_asserts=False,
        num_devices=NCORES,
    )

    hT_d = nc.dram_tensor("hT", [C, N], F32, kind="ExternalInput")
    hTi_d = nc.dram_tensor("hTi", [C, ROWS], F32, kind="ExternalInput")
    WT_d = nc.dram_tensor("WT", [C, C], F32, kind="ExternalInput")
    bgb_d = nc.dram_tensor("bgb", [128, 3 * C], F32, kind="ExternalInput")
    sel_d = nc.dram_tensor("sel", [2 * C, C * 128], BF16, kind="ExternalInput")
    id_d = nc.dram_tensor("identity", [128, 128], F32, kind="ExternalInput")
    out_d = nc.dram_tensor("out", [ROWS, N * C], F32, kind="ExternalOutput")

    X = mybir.AxisListType.X
    sub = mybir.AluOpType.subtract
    mult = mybir.AluOpType.mult
    Exp = mybir.ActivationFunctionType.Exp
    Sqrt = mybir.ActivationFunctionType.Sqrt

    with tile.TileContext(nc) as tc:
        with tc.tile_pool(name="const", bufs=1) as constp:
            hTi = constp.tile([C, ROWS], F32)
            nc.sync.dma_start(hTi[:], hTi_d[:])
            WT = constp.tile([C, C], F32)
            nc.sync.dma_start(WT[:], WT_d[:])
            bgb = constp.tile([128, 3 * C], F32)
            nc.sync.dma_start(bgb[:], bgb_d[:])
            ident = constp.tile([128, 128], F32)
            nc.sync.dma_start(ident[:], id_d[:])
            hT = constp.tile([C, N], F32)
            nc.sync.dma_start(hT[:], hT_d[:])
            # sel is 2 MB and only needed by the first broadcast matmul;
            # load it via the SWDGE path so it doesn't queue ahead of the
            # small prep inputs on the HWDGE FIFO.
            sel = constp.tile([2 * C, C * 128], BF16)
            nc.gpsimd.dma_start(sel[:], sel_d[:])

            xT = constp.tile([C, N], F32)          # x[b].T  (c on partitions)
            xi = constp.tile([128, 2, C], F32)     # this core's two i-tiles
            xT_hilo = constp.tile([128, N], BF16)  # K-stacked bf16 hi/lo of xT
            eps_t = constp.tile([128, 1], F32)
            nc.vector.memset(eps_t[:], EPS)

            # ---- x = LayerNorm(ReLU(h @ W.T + b)) --------------------------
            # i-tiles (t=4,5) first so the main loop's scale operand is ready
            # early; each xT slice gets its bf16 hi/lo split as soon as it is
            # transposed, letting the first broadcast matmuls start before
            # the whole prep finishes.
            with (
                tc.tile_pool(name="xprep", bufs=2) as xprep,
                tc.tile_pool(name="psum_prep", bufs=2, space=bass.MemorySpace.PSUM) as psp,
                tc.tile_pool(name="psum_tp", bufs=2, space=bass.MemorySpace.PSUM) as ptp,
            ):
                for t in (4, 5, 0, 1, 2, 3):
                    if t < 4:
                        lhsT = hT[:, t * 128:(t + 1) * 128]
                    else:
                        lhsT = hTi[:, (t - 4) * 128:(t - 3) * 128]
                    xp = psp.tile([128, C], F32, tag="xp")
                    nc.tensor.matmul(xp[:], lhsT, WT[:])
                    xs = xprep.tile([128, C], F32, tag="xs")
                    nc.vector.tensor_add(xs[:], xp[:], bgb[:, 0:C])       # + b
                    nc.scalar.activation(
                        xs[:], xs[:], mybir.ActivationFunctionType.Relu
                    )
                    stats = xprep.tile([128, 6], F32, tag="stats")
                    nc.vector.bn_stats(stats[:], xs[:])
                    mv = xprep.tile([128, 2], F32, tag="mv")
                    nc.vector.bn_aggr(mv[:], stats[:])
                    std = xprep.tile([128, 1], F32, tag="std")
                    nc.scalar.activation(std[:], mv[:, 1:2], Sqrt, bias=eps_t[:, 0:1])
                    rstd = xprep.tile([128, 1], F32, tag="rstd")
                    nc.vector.reciprocal(rstd[:], std[:])
                    xn = xprep.tile([128, C], F32, tag="xn")
                    nc.vector.tensor_scalar(
                        xn[:], xs[:], mv[:, 0:1], rstd[:, 0:1], op0=sub, op1=mult
                    )
                    nc.vector.tensor_mul(xn[:], xn[:], bgb[:, C:2 * C])   # * gamma
                    nc.vector.tensor_add(xn[:], xn[:], bgb[:, 2 * C:3 * C])  # + beta
                    if t < 4:
                        tp = ptp.tile([C, 128], F32, tag="tp")
                        nc.tensor.transpose(tp[:], xn[:], ident[:])
                        sl = slice(t * 128, (t + 1) * 128)
                        nc.vector.tensor_copy(xT[:, sl], tp[:])
                        # K-stacked bf16 hi/lo split of this slice:
                        # partitions 0-63 hold bf16(x), 64-127 bf16(x - hi).
                        # One K=128 matmul with the doubled selector then
                        # sums both rank-64 halves in PSUM fp32, reproducing
                        # the f32 broadcast exactly to ~2^-17 in one pass.
                        nc.vector.tensor_copy(xT_hilo[0:C, sl], xT[:, sl])
                        hi32 = xprep.tile([C, 128], F32, tag="hi32")
                        nc.vector.tensor_copy(hi32[:], xT_hilo[0:C, sl])
                        nc.vector.tensor_sub(xT_hilo[C:2 * C, sl], xT[:, sl], hi32[:])
                    else:
                        nc.vector.tensor_copy(xi[:, t - 4, :], xn[:])

            # ---- main: exp(x_i * x_j), softmax over c, store ---------------
            # Chunk widths are asymmetric: a narrow first chunk lets the
            # vector engine (the critical path) start early, and a narrow
            # last chunk shrinks the drain tail. Still 2 activation
            # instructions per (i-tile, c), so ScalarE time is unchanged.
            CHUNKS = {0: (224, 288), 1: (288, 224)}
            with (
                tc.tile_pool(name="main", bufs=2) as mainp,
                tc.tile_pool(name="small", bufs=3) as smallp,
                tc.tile_pool(name="psum_bc", bufs=6, space=bass.MemorySpace.PSUM) as pbc,
            ):
                chunk_idx = 0
                for it in range(2):
                    j0 = 0
                    for jw in CHUNKS[it]:
                        expt = mainp.tile([128, jw, C], F32, tag="exp")
                        for c in range(C):
                            bc = pbc.tile([128, jw], F32, tag="bc")
                            nc.tensor.matmul(
                                bc[:],
                                sel[:, c * 128:(c + 1) * 128],
                                xT_hilo[:, j0:j0 + jw],
                            )
                            nc.scalar.activation(
                                expt[:, :, c], bc[:], Exp, scale=xi[:, it, c:c + 1]
                            )
                        # Segmented sum over c. For the first two chunks the
                        # reduce is split into c-groups so VectorE can start
                        # on partial sums while ScalarE is still producing
                        # the remaining channels (hides the pipeline head);
                        # later chunks use a single reduce (vector is the
                        # busy engine there, partials would only add cost).
                        sums = smallp.tile([128, jw], F32, tag="sums")
                        if chunk_idx < 3:
                            part = smallp.tile([128, jw], F32, tag="part")
                            for g in range(4):
                                csl = slice(g * 16, (g + 1) * 16)
                                tgt = sums if g == 0 else part
                                nc.vector.reduce_sum(
                                    tgt[:], expt[:, :, csl], axis=X
                                )
                                if g > 0:
                                    nc.vector.tensor_add(sums[:], sums[:], part[:])
                        else:
                            nc.vector.reduce_sum(sums[:], expt[:], axis=X)
                        recip = smallp.tile([128, jw], F32, tag="recip")
                        nc.vector.reciprocal(recip[:], sums[:])
                        # normalize in j-slices; each slice DMAs out as
                        # soon as it is scaled (frees the exp buffer sooner
                        # and overlaps store with compute). The final chunk
                        # uses finer slices to shrink the drain tail.
                        NS = 8 if chunk_idx == 3 else 4
                        QW = jw // NS
                        for q in range(NS):
                            sl = slice(q * QW, (q + 1) * QW)
                            nc.vector.tensor_mul(
                                expt[:, sl, :],
                                expt[:, sl, :],
                                recip[:, sl][:, :, None].broadcast_to((128, QW, C)),
                            )
                            nc.sync.dma_start(
                                out_d[it * 128:(it + 1) * 128,
                                      (j0 + q * QW) * C:(j0 + (q + 1) * QW) * C],
                                expt[:, sl, :].rearrange("p j c -> p (j c)"),
                            )
                        j0 += jw
                        chunk_idx += 1
    nc.compile()
    return nc


def _in_maps(h, W, b, gamma, beta):
    h = np.asarray(h, dtype=np.float32)
    W = np.asarray(W, dtype=np.float32)
    b = np.asarray(b, dtype=np.float32)
    gamma = np.asarray(gamma, dtype=np.float32)
    beta = np.asarray(beta, dtype=np.float32)

    WT = np.ascontiguousarray(W.T)
    bgb = np.ascontiguousarray(
        np.broadcast_to(np.concatenate([b, gamma, beta])[None, :], (128, 3 * C))
    )
    import ml_dtypes
    sel = np.zeros((2 * C, C * 128), dtype=ml_dtypes.bfloat16)
    for c in range(C):
        sel[c, c * 128:(c + 1) * 128] = 1.0
        sel[C + c, c * 128:(c + 1) * 128] = 1.0
    ident = np.eye(128, dtype=np.float32)

    in_maps = []
    for k in range(NCORES):
        bb, half = divmod(k, 2)
        i0 = half * ROWS
        in_maps.append({
            "hT": np.ascontiguousarray(h[bb].T),
            "hTi": np.ascontiguousarray(h[bb, i0:i0 + ROWS].T),
            "WT": WT,
            "bgb": bgb,
            "sel": sel,
            "identity": ident,
        })
    return in_maps


def run(h, W, b, gamma, beta, trace=False, **trace_kwargs):
    if "nc" not in _CACHE:
        _CACHE["nc"] = _build_program()
    nc = _CACHE["nc"]
    res = run_bass_kernel_spmd(
        nc,
        _in_maps(h, W, b, gamma, beta),
        core_ids=list(range(NCORES)),
        trace=trace,
        **trace_kwargs,
    )
    out = np.zeros((B, N, N, C), dtype=np.float32)
    for k in range(NCORES):
        bb, half = divmod(k, 2)
        i0 = half * ROWS
        out[bb, i0:i0 + ROWS] = res.results[k]["out"].reshape(ROWS, N, C)
    return out, res


def kernel(h, W, b, gamma, beta):
    out, _ = run(h, W, b, gamma, beta)
    return out


# revision 34
# speedup vs baseline: 1.0251x; 1.0251x over previous
"""Trainium2 Bass kernel for nn_DistanceModule.

Computes, for h [4,512,64], W [64,64], b/gamma/beta [64]:
    x = LayerNorm(ReLU(h @ W.T + b))          # [B,N,C]
    D[b,i,j,c] = x[b,i,c] * x[b,j,c]
    out = softmax(D, axis=-1)                 # [B,N,N,C] f32 (256 MB)

Sharding: 2048 (b,i) rows split across 8 cores -> 256 rows/core
(core k: batch b=k//2, i in [256*(k%2), 256*(k%2)+256)). Each core
computes x[b] on-chip, then streams its [256, 512, 64] output slice.
All cores run one identical NEFF; per-core behavior comes only from
per-core input slices (hT = h[b].T, hTi = h[b, i0:i0+256].T).

Per-core pipeline, per (i-tile, j-chunk), all engines overlapped:
  PE     : one K=128 bf16 matmul per channel c broadcasts xT row c
           across 128 partitions into PSUM. The K axis stacks an exact
           hi/lo bf16 split of xT (x = hi + lo to ~2^-17) against a
           doubled 0/1 selector, so fp32 accuracy is reproduced at bf16
           matmul speed in a single pass.
  ScalarE: activation(Exp, scale=x_i[:,c]) reads the PSUM broadcast and
           fuses the x_i*x_j multiply into the exp via the per-partition
           scale operand -- one FD=jw instruction per (i-tile, c).
  VectorE: segmented reduce_sum over c (axis=X on the [128, j, c] view),
           reciprocal, then in-place normalize multiply against a
           stride-0-broadcast reciprocal AP. This engine is the
           critical path (~1 elem/lane/cycle for reduce and multiply).
  DMA    : normalize runs in j-quarters, each immediately stored with a
           contiguous 128-partition HWDGE DMA (64KB/partition rows).

Chunk widths (224/288) keep ScalarE's per-instruction overhead (~400
cycles, from the per-partition bias+scale preloads) balanced against
VectorE's chunk time, and a narrow first/last chunk trims the pipeline
fill/drain.

Softmax is computed without max-subtraction: LayerNorm bounds |x| by
sqrt(C-1) ~= 7.94, so logits <= 63 and exp <= 2.4e27 < f32 max.
Measured: ~228 us HW exec, rel err ~5e-6 vs the f32 reference.
"""

import numpy as np

import concourse.bacc as bacc
import concourse.bass as bass
import concourse.mybir as mybir
import concourse.tile as tile
from concourse.bass_utils import run_bass_kernel_spmd

B, N, C = 4, 512, 64
NCORES = 8
ROWS = 256          # (b,i) rows per core
JBLK = 256          # j-block width
EPS = 1e-5
F32 = mybir.dt.float32
BF16 = mybir.dt.bfloat16

_CACHE = {}


def _build_program():
    nc = bacc.Bacc(
        "TRN2",
        target_bir_lowering=False,
        debug=False,
        enable_asserts=False,
        num_devices=NCORES,
    )

    hT_d = nc.dram_tensor("hT", [C, N], F32, kind="ExternalInput")
    hTi_d = nc.dram_tensor("hTi", [C, ROWS], F32, kind="ExternalInput")
    WT_d = nc.dram_tensor("WT", [C, C], F32, kind="ExternalInput")
    bgb_d = nc.dram_tensor("bgb", [128, 3 * C], F32, kind="ExternalInput")
    sel_d = nc.dram_tensor("sel", [2 * C, C * 128], BF16, kind="ExternalInput")
    id_d = nc.dram_tensor("identity", [128, 128], F32, kind="ExternalInput")
    out_d = nc.dram_tensor("out", [ROWS, N * C], F32, kind="ExternalOutput")

    X = mybir.AxisListType.X
    sub = mybir.AluOpType.subtract
    mult = mybir.AluOpType.mult
    Exp = mybir.ActivationFunctionType.Exp
    Sqrt = mybir.ActivationFunctionType.Sqrt

    with tile.TileContext(nc) as tc:
        with tc.tile_pool(name="const", bufs=1) as constp:
            hTi = constp.tile([C, ROWS], F32)
            nc.sync.dma_start(hTi[:], hTi_d[:])
            hT = constp.tile([C, N], F32)
            nc.sync.dma_start(hT[:], hT_d[:])
            WT = constp.tile([C, C], F32)
            nc.sync.dma_start(WT[:], WT_d[:])
            bgb = constp.tile([128, 3 * C], F32)
            nc.sync.dma_start(bgb[:], bgb_d[:])
            sel = constp.tile([2 * C, C * 128], BF16)
            nc.sync.dma_start(sel[:], sel_d[:])
            ident = constp.tile([128, 128], F32)
            nc.sync.dma_start(ident[:], id_d[:])

            xT = constp.tile([C, N], F32)          # x[b].T  (c on partitions)
            xi = constp.tile([128, 2, C], F32)     # this core's two i-tiles
            xT_hilo = constp.tile([128, N], BF16)  # K-stacked bf16 hi/lo of xT
            eps_t = constp.tile([128, 1], F32)
            nc.vector.memset(eps_t[:], EPS)

            # ---- x = LayerNorm(ReLU(h @ W.T + b)) --------------------------
            # i-tiles (t=4,5) first so the main loop's scale operand is ready
            # early; each xT slice gets its bf16 hi/lo split as soon as it is
            # transposed, letting the first broadcast matmuls start before
            # the whole prep finishes.
            with (
                tc.tile_pool(name="xprep", bufs=2) as xprep,
                tc.tile_pool(name="psum_prep", bufs=2, space=bass.MemorySpace.PSUM) as psp,
                tc.tile_pool(name="psum_tp", bufs=2, space=bass.MemorySpace.PSUM) as ptp,
            ):
                for t in (4, 5, 0, 1, 2, 3):
                    if t < 4:
                        lhsT = hT[:, t * 128:(t + 1) * 128]
                    else:
                        lhsT = hTi[:, (t - 4) * 128:(t - 3) * 128]
                    xp = psp.tile([128, C], F32, tag="xp")
                    nc.tensor.matmul(xp[:], lhsT, WT[:])
                    xs = xprep.tile([128, C], F32, tag="xs")
                    nc.vector.tensor_add(xs[:], xp[:], bgb[:, 0:C])       # + b
                    nc.scalar.activation(
                        xs[:], xs[:], mybir.ActivationFunctionType.Relu
                    )
                    stats = xprep.tile([128, 6], F32, tag="stats")
                    nc.vector.bn_stats(stats[:], xs[:])
                    mv = xprep.tile([128, 2], F32, tag="mv")
                    nc.vector.bn_aggr(mv[:], stats[:])
                    std = xprep.tile([128, 1], F32, tag="std")
                    nc.scalar.activation(std[:], mv[:, 1:2], Sqrt, bias=eps_t[:, 0:1])
                    rstd = xprep.tile([128, 1], F32, tag="rstd")
                    nc.vector.reciprocal(rstd[:], std[:])
                    xn = xprep.tile([128, C], F32, tag="xn")
                    nc.vector.tensor_scalar(
                        xn[:], xs[:], mv[:, 0:1], rstd[:, 0:1], op0=sub, op1=mult
                    )
                    nc.vector.tensor_mul(xn[:], xn[:], bgb[:, C:2 * C])   # * gamma
                    nc.vector.tensor_add(xn[:], xn[:], bgb[:, 2 * C:3 * C])  # + beta
                    if t < 4:
                        tp = ptp.tile([C, 128], F32, tag="tp")
                        nc.tensor.transpose(tp[:], xn[:], ident[:])
                        sl = slice(t * 128, (t + 1) * 128)
                        nc.vector.tensor_copy(xT[:, sl], tp[:])
                        # K-stacked bf16 hi/lo split of this slice:
                        # partitions 0-63 hold bf16(x), 64-127 bf16(x - hi).
                        # One K=128 matmul with the doubled selector then
                        # sums both rank-64 halves in PSUM fp32, reproducing
                        # the f32 broadcast exactly to ~2^-17 in one pass.
                        nc.vector.tensor_copy(xT_hilo[0:C, sl], xT[:, sl])
                        hi32 = xprep.tile([C, 128], F32, tag="hi32")
                        nc.vector.tensor_copy(hi32[:], xT_hilo[0:C, sl])
                        nc.vector.tensor_sub(xT_hilo[C:2 * C, sl], xT[:, sl], hi32[:])
                    else:
                        nc.vector.tensor_copy(xi[:, t - 4, :], xn[:])

            # ---- main: exp(x_i * x_j), softmax over c, store ---------------
            # Chunk widths are asymmetric: a narrow first chunk lets the
            # vector engine (the critical path) start early, and a narrow
            # last chunk shrinks the drain tail. Still 2 activation
            # instructions per (i-tile, c), so ScalarE time is unchanged.
            CHUNKS = {0: (224, 288), 1: (288, 224)}
            with (
                tc.tile_pool(name="main", bufs=2) as mainp,
                tc.tile_pool(name="small", bufs=3) as smallp,
                tc.tile_pool(name="psum_bc", bufs=6, space=bass.MemorySpace.PSUM) as pbc,
            ):
                chunk_idx = 0
                for it in range(2):
                    j0 = 0
                    for jw in CHUNKS[it]:
                        expt = mainp.tile([128, jw, C], F32, tag="exp")
                        for c in range(C):
                            bc = pbc.tile([128, jw], F32, tag="bc")
                            nc.tensor.matmul(
                                bc[:],
                                sel[:, c * 128:(c + 1) * 128],
                                xT_hilo[:, j0:j0 + jw],
                            )
                            nc.scalar.activation(
                                expt[:, :, c], bc[:], Exp, scale=xi[:, it, c:c + 1]
                            )
                        # Segmented sum over c. For the first two chunks the
                        # reduce is split into c-groups so VectorE can start
                        # on partial sums while ScalarE is still producing
                        # the remaining channels (hides the pipeline head);
                        # later chunks use a single reduce (vector is the
                        # busy engine there, partials would only add cost).
                        sums = smallp.tile([128, jw], F32, tag="sums")
                        if chunk_idx < 3:
                            part = smallp.tile([128, jw], F32, tag="part")
                            for g in range(4):
                                csl = slice(g * 16, (g + 1) * 16)
                                tgt = sums if g == 0 else part
                                nc.vector.reduce_sum(
                                    tgt[:], expt[:, :, csl], axis=X
                                )
                                if g > 0:
                                    nc.vector.tensor_add(sums[:], sums[:], part[:])
                        else:
                            nc.vector.reduce_sum(sums[:], expt[:], axis=X)
                        recip = smallp.tile([128, jw], F32, tag="recip")
                        nc.vector.reciprocal(recip[:], sums[:])
                        # normalize in j-quarters; each quarter DMAs out as
                        # soon as it is scaled (frees the exp buffer sooner
                        # and overlaps store with compute).
                        QW = jw // 4
                        for q in range(4):
                            sl = slice(q * QW, (q + 1) * QW)
                            nc.vector.tensor_mul(
                                expt[:, sl, :],
                                expt[:, sl, :],
                                recip[:, sl][:, :, None].broadcast_to((128, QW, C)),
                            )
                            nc.sync.dma_start(
                                out_d[it * 128:(it + 1) * 128,
                                      (j0 + q * QW) * C:(j0 + (q + 1) * QW) * C],
                                expt[:, sl, :].rearrange("p j c -> p (j c)"),
                            )
                        j0 += jw
                        chunk_idx += 1
    nc.compile()
    return nc


def _in_maps(h, W, b, gamma, beta):
    h = np.asarray(h, dtype=np.float32)
    W = np.asarray(W, dtype=np.float32)
    b = np.asarray(b, dtype=np.float32)
    gamma = np.asarray(gamma, dtype=np.float32)
    beta = np.asarray(beta, dtype=np.float32)

    WT = np.ascontiguousarray(W.T)
    bgb = np.ascontiguousarray(
        np.broadcast_to(np.concatenate([b, gamma, beta])[None, :], (128, 3 * C))
    )
    import ml_dtypes
    sel = np.zeros((2 * C, C * 128), dtype=ml_dtypes.bfloat16)
    for c in range(C):
        sel[c, c * 128:(c + 1) * 128] = 1.0
        sel[C + c, c * 128:(c + 1) * 128] = 1.0
    ident = np.eye(128, dtype=np.float32)

    in_maps = []
    for k in range(NCORES):
        bb, half = divmod(k, 2)
        i0 = half * ROWS
        in_maps.append({
            "hT": np.ascontiguousarray(h[bb].T),
            "hTi": np.ascontiguousarray(h[bb, i0:i0 + ROWS].T),
            "WT": WT,
            "bgb": bgb,
            "sel": sel,
            "identity": ident,
        })
    return in_maps


def run(h, W, b, gamma, beta, trace=False, **trace_kwargs):
    if "nc" not in _CACHE:
        _CACHE["nc"] = _build_program()
    nc = _CACHE["nc"]
    res = run_bass_kernel_spmd(
        nc,
        _in_maps(h, W, b, gamma, beta),
        core_ids=list(range(NCORES)),
        trace=trace,
        **trace_kwargs,
    )
    out = np.zeros((B, N, N, C), dtype=np.float32)
    for k in range(NCORES):
        bb, half = divmod(k, 2)
        i0 = half * ROWS
        out[bb, i0:i0 + ROWS] = res.results[k]["out"].reshape(ROWS, N, C)
    return out, res


def kernel(h, W, b, gamma, beta):
    out, _ = run(h, W, b, gamma, beta)
    return out


# revision 35
# speedup vs baseline: 1.0729x; 1.0466x over previous
"""Trainium2 Bass kernel for nn_DistanceModule.

Computes, for h [4,512,64], W [64,64], b/gamma/beta [64]:
    x = LayerNorm(ReLU(h @ W.T + b))          # [B,N,C]
    D[b,i,j,c] = x[b,i,c] * x[b,j,c]
    out = softmax(D, axis=-1)                 # [B,N,N,C] f32 (256 MB)

Sharding: 2048 (b,i) rows split across 8 cores -> 256 rows/core
(core k: batch b=k//2, i in [256*(k%2), 256*(k%2)+256)). Each core
computes x[b] on-chip, then streams its [256, 512, 64] output slice.
All cores run one identical NEFF; per-core behavior comes only from
per-core input slices (hT = h[b].T, hTi = h[b, i0:i0+256].T).

Per-core pipeline, per (i-tile, j-chunk), all engines overlapped:
  PE     : one K=128 bf16 matmul per channel c broadcasts xT row c
           across 128 partitions into PSUM. The K axis stacks an exact
           hi/lo bf16 split of xT (x = hi + lo to ~2^-17) against a
           doubled 0/1 selector, so fp32 accuracy is reproduced at bf16
           matmul speed in a single pass.
  ScalarE: activation(Exp, scale=x_i[:,c]) reads the PSUM broadcast and
           fuses the x_i*x_j multiply into the exp via the per-partition
           scale operand -- one FD=jw instruction per (i-tile, c).
  VectorE: segmented reduce_sum over c (axis=X on the [128, j, c] view),
           reciprocal, then in-place normalize multiply against a
           stride-0-broadcast reciprocal AP. This engine is the
           critical path (~1 elem/lane/cycle for reduce and multiply).
  DMA    : normalize runs in j-quarters, each immediately stored with a
           contiguous 128-partition HWDGE DMA (64KB/partition rows).

Chunk widths (224/288) keep ScalarE's per-instruction overhead (~400
cycles, from the per-partition bias+scale preloads) balanced against
VectorE's chunk time, and a narrow first/last chunk trims the pipeline
fill/drain.

Softmax is computed without max-subtraction: LayerNorm bounds |x| by
sqrt(C-1) ~= 7.94, so logits <= 63 and exp <= 2.4e27 < f32 max.
Measured: ~228 us HW exec, rel err ~5e-6 vs the f32 reference.
"""

import numpy as np

import concourse.bacc as bacc
import concourse.bass as bass
import concourse.mybir as mybir
import concourse.tile as tile
from concourse.bass_utils import run_bass_kernel_spmd

B, N, C = 4, 512, 64
NCORES = 8
ROWS = 256          # (b,i) rows per core
JBLK = 256          # j-block width
EPS = 1e-5
F32 = mybir.dt.float32
BF16 = mybir.dt.bfloat16

_CACHE = {}


def _build_program():
    nc = bacc.Bacc(
        "TRN2",
        target_bir_lowering=False,
        debug=False,
        enable_asserts=False,
        num_devices=NCORES,
    )

    hT_d = nc.dram_tensor("hT", [C, N], F32, kind="ExternalInput")
    hTi_d = nc.dram_tensor("hTi", [C, ROWS], F32, kind="ExternalInput")
    WT_d = nc.dram_tensor("WT", [C, C], F32, kind="ExternalInput")
    bgb_d = nc.dram_tensor("bgb", [128, 3 * C], F32, kind="ExternalInput")
    sel_d = nc.dram_tensor("sel", [2 * C, C * 128], BF16, kind="ExternalInput")
    id_d = nc.dram_tensor("identity", [128, 128], F32, kind="ExternalInput")
    out_d = nc.dram_tensor("out", [ROWS, N * C], F32, kind="ExternalOutput")

    X = mybir.AxisListType.X
    sub = mybir.AluOpType.subtract
    mult = mybir.AluOpType.mult
    Exp = mybir.ActivationFunctionType.Exp
    Sqrt = mybir.ActivationFunctionType.Sqrt

    with tile.TileContext(nc) as tc:
        with tc.tile_pool(name="const", bufs=1) as constp:
            hTi = constp.tile([C, ROWS], F32)
            nc.sync.dma_start(hTi[:], hTi_d[:])
            hT = constp.tile([C, N], F32)
            nc.sync.dma_start(hT[:], hT_d[:])
            WT = constp.tile([C, C], F32)
            nc.sync.dma_start(WT[:], WT_d[:])
            bgb = constp.tile([128, 3 * C], F32)
            nc.sync.dma_start(bgb[:], bgb_d[:])
            sel = constp.tile([2 * C, C * 128], BF16)
            nc.sync.dma_start(sel[:], sel_d[:])
            ident = constp.tile([128, 128], F32)
            nc.sync.dma_start(ident[:], id_d[:])

            xT = constp.tile([C, N], F32)          # x[b].T  (c on partitions)
            xi = constp.tile([128, 2, C], F32)     # this core's two i-tiles
            xT_hilo = constp.tile([128, N], BF16)  # K-stacked bf16 hi/lo of xT
            eps_t = constp.tile([128, 1], F32)
            nc.vector.memset(eps_t[:], EPS)

            # ---- x = LayerNorm(ReLU(h @ W.T + b)) --------------------------
            # i-tiles (t=4,5) first so the main loop's scale operand is ready
            # early; each xT slice gets its bf16 hi/lo split as soon as it is
            # transposed, letting the first broadcast matmuls start before
            # the whole prep finishes.
            with (
                tc.tile_pool(name="xprep", bufs=2) as xprep,
                tc.tile_pool(name="psum_prep", bufs=2, space=bass.MemorySpace.PSUM) as psp,
                tc.tile_pool(name="psum_tp", bufs=2, space=bass.MemorySpace.PSUM) as ptp,
            ):
                for t in (4, 5, 0, 1, 2, 3):
                    if t < 4:
                        lhsT = hT[:, t * 128:(t + 1) * 128]
                    else:
                        lhsT = hTi[:, (t - 4) * 128:(t - 3) * 128]
                    xp = psp.tile([128, C], F32, tag="xp")
                    nc.tensor.matmul(xp[:], lhsT, WT[:])
                    xs = xprep.tile([128, C], F32, tag="xs")
                    nc.vector.tensor_add(xs[:], xp[:], bgb[:, 0:C])       # + b
                    nc.scalar.activation(
                        xs[:], xs[:], mybir.ActivationFunctionType.Relu
                    )
                    stats = xprep.tile([128, 6], F32, tag="stats")
                    nc.vector.bn_stats(stats[:], xs[:])
                    mv = xprep.tile([128, 2], F32, tag="mv")
                    nc.vector.bn_aggr(mv[:], stats[:])
                    std = xprep.tile([128, 1], F32, tag="std")
                    nc.scalar.activation(std[:], mv[:, 1:2], Sqrt, bias=eps_t[:, 0:1])
                    rstd = xprep.tile([128, 1], F32, tag="rstd")
                    nc.vector.reciprocal(rstd[:], std[:])
                    xn = xprep.tile([128, C], F32, tag="xn")
                    nc.vector.tensor_scalar(
                        xn[:], xs[:], mv[:, 0:1], rstd[:, 0:1], op0=sub, op1=mult
                    )
                    nc.vector.tensor_mul(xn[:], xn[:], bgb[:, C:2 * C])   # * gamma
                    nc.vector.tensor_add(xn[:], xn[:], bgb[:, 2 * C:3 * C])  # + beta
                    if t < 4:
                        tp = ptp.tile([C, 128], F32, tag="tp")
                        nc.tensor.transpose(tp[:], xn[:], ident[:])
                        sl = slice(t * 128, (t + 1) * 128)
                        nc.vector.tensor_copy(xT[:, sl], tp[:])
                        # K-stacked bf16 hi/lo split of this slice:
                        # partitions 0-63 hold bf16(x), 64-127 bf16(x - hi).
                        # One K=128 matmul with the doubled selector then
                        # sums both rank-64 halves in PSUM fp32, reproducing
                        # the f32 broadcast exactly to ~2^-17 in one pass.
                        nc.vector.tensor_copy(xT_hilo[0:C, sl], xT[:, sl])
                        hi32 = xprep.tile([C, 128], F32, tag="hi32")
                        nc.vector.tensor_copy(hi32[:], xT_hilo[0:C, sl])
                        nc.vector.tensor_sub(xT_hilo[C:2 * C, sl], xT[:, sl], hi32[:])
                    else:
                        nc.vector.tensor_copy(xi[:, t - 4, :], xn[:])

            # ---- main: exp(x_i * x_j), softmax over c, store ---------------
            # Chunk widths are asymmetric: a narrow first chunk lets the
            # vector engine (the critical path) start early, and a narrow
            # last chunk shrinks the drain tail. Still 2 activation
            # instructions per (i-tile, c), so ScalarE time is unchanged.
            CHUNKS = {0: (224, 288), 1: (288, 224)}
            with (
                tc.tile_pool(name="main", bufs=2) as mainp,
                tc.tile_pool(name="small", bufs=3) as smallp,
                tc.tile_pool(name="psum_bc", bufs=6, space=bass.MemorySpace.PSUM) as pbc,
            ):
                chunk_idx = 0
                for it in range(2):
                    j0 = 0
                    for jw in CHUNKS[it]:
                        expt = mainp.tile([128, jw, C], F32, tag="exp")
                        for c in range(C):
                            bc = pbc.tile([128, jw], F32, tag="bc")
                            nc.tensor.matmul(
                                bc[:],
                                sel[:, c * 128:(c + 1) * 128],
                                xT_hilo[:, j0:j0 + jw],
                            )
                            nc.scalar.activation(
                                expt[:, :, c], bc[:], Exp, scale=xi[:, it, c:c + 1]
                            )
                        # Segmented sum over c. For the first two chunks the
                        # reduce is split into c-groups so VectorE can start
                        # on partial sums while ScalarE is still producing
                        # the remaining channels (hides the pipeline head);
                        # later chunks use a single reduce (vector is the
                        # busy engine there, partials would only add cost).
                        sums = smallp.tile([128, jw], F32, tag="sums")
                        if chunk_idx < 2:
                            part = smallp.tile([128, jw], F32, tag="part")
                            for g in range(4):
                                csl = slice(g * 16, (g + 1) * 16)
                                tgt = sums if g == 0 else part
                                nc.vector.reduce_sum(
                                    tgt[:], expt[:, :, csl], axis=X
                                )
                                if g > 0:
                                    nc.vector.tensor_add(sums[:], sums[:], part[:])
                        else:
                            nc.vector.reduce_sum(sums[:], expt[:], axis=X)
                        recip = smallp.tile([128, jw], F32, tag="recip")
                        nc.vector.reciprocal(recip[:], sums[:])
                        # normalize in j-quarters; each quarter DMAs out as
                        # soon as it is scaled (frees the exp buffer sooner
                        # and overlaps store with compute).
                        QW = jw // 4
                        for q in range(4):
                            sl = slice(q * QW, (q + 1) * QW)
                            nc.vector.tensor_mul(
                                expt[:, sl, :],
                                expt[:, sl, :],
                                recip[:, sl][:, :, None].broadcast_to((128, QW, C)),
                            )
                            nc.sync.dma_start(
                                out_d[it * 128:(it + 1) * 128,
                                      (j0 + q * QW) * C:(j0 + (q + 1) * QW) * C],
                                expt[:, sl, :].rearrange("p j c -> p (j c)"),
                            )
                        j0 += jw
                        chunk_idx += 1
    nc.compile()
    return nc


def _in_maps(h, W, b, gamma, beta):
    h = np.asarray(h, dtype=np.float32)
    W = np.asarray(W, dtype=np.float32)
    b = np.asarray(b, dtype=np.float32)
    gamma = np.asarray(gamma, dtype=np.float32)
    beta = np.asarray(beta, dtype=np.float32)

    WT = np.ascontiguousarray(W.T)
    bgb = np.ascontiguousarray(
        np.broadcast_to(np.concatenate([b, gamma, beta])[None, :], (128, 3 * C))
    )
    import ml_dtypes
    sel = np.zeros((2 * C, C * 128), dtype=ml_dtypes.bfloat16)
    for c in range(C):
        sel[c, c * 128:(c + 1) * 128] = 1.0
        sel[C + c, c * 128:(c + 1) * 128] = 1.0
    ident = np.eye(128, dtype=np.float32)

    in_maps = []
    for k in range(NCORES):
        bb, half = divmod(k, 2)
        i0 = half * ROWS
        in_maps.append({
            "hT": np.ascontiguousarray(h[bb].T),
            "hTi": np.ascontiguousarray(h[bb, i0:i0 + ROWS].T),
            "WT": WT,
            "bgb": bgb,
            "sel": sel,
            "identity": ident,
        })
    return in_maps


def run(h, W, b, gamma, beta, trace=False, **trace_kwargs):
    if "nc" not in _CACHE:
        _CACHE["nc"] = _build_program()
    nc = _CACHE["nc"]
    res = run_bass_kernel_spmd(
        nc,
        _in_maps(h, W, b, gamma, beta),
        core_ids=list(range(NCORES)),
        trace=trace,
        **trace_kwargs,
    )
    out = np.zeros((B, N, N, C), dtype=np.float32)
    for k in range(NCORES):
        bb, half = divmod(k, 2)
        i0 = half * ROWS
        out[bb, i0:i0 + ROWS] = res.results[k]["out"].reshape(ROWS, N, C)
    return out, res


def kernel(h, W, b, gamma, beta):
    out, _ = run(h, W, b, gamma, beta)
    return out
